# revision 18
# baseline (speedup 1.0000x reference)
"""Dynamic per-sample CNN (nn_ConvFunc) Trainium2 Bass kernel.

Reference computation (per sample b):
  cnn_inp = proj_w @ cat(lhs, rhs) + proj_b          # 1x1 conv, [128, 32, 32]
  out     = conv3x3(cnn_inp, W_b) + bias_b           # W_b, bias_b unpacked from question_rep[b]

Sharding: pure data parallel, 8 samples per NeuronCore (batch 64 / 8 cores).

All matmul operands are bf16 (tolerance 2e-2; measured rel err ~4e-3), which
halves HBM traffic vs fp32 and streams 1 column/cycle on the PE. Output is
stored bf16 and widened to fp32 on host (exact widening).

Layout: xc per sample is [xl-h0 | xr-h0 | xl-h1 | xr-h1] in 512-column
blocks, so each proj half needs one contiguous half of the tensor and the
two accumulating matmuls read adjacent blocks.

Schedule: the PE instruction order is pinned with tile_wait_until virtual
times. The DMA delivery ramp (~100->400 GB/s over the first ~8us) cannot
feed lookahead at sample 0, so the fill runs proj(0) -> conv(0) directly
(conv needs 4x less bandwidth per PE-second than proj) with warmup matmuls
bridging the PSUM eviction latency; from s=1 on, the steady software
pipeline proj(s+1)-then-conv(s) hides eviction latency for free. Warmup
matmuls also run from ~7us during the first DMAs, lifting the PE HAM clock
gate (4/8 -> 8/8) before the first real matmul. Engine assignment avoids
static-queue-order inversions across the pipeline: DVE owns proj evictions
(+pad memsets), ACT owns conv evictions + stores, sync/scalar queues issue
the loads.
"""

import numpy as np
import ml_dtypes

import concourse.bass as bass
import concourse.mybir as mybir
from concourse import bacc
from concourse.tile import TileContext
from concourse.bass_utils import run_bass_kernel_spmd

# Problem shapes (hardcoded per contract)
B = 64
DIM = 128
H = W = 32
K = 3
KK = K * K
HW = H * W             # 1024
WDIM = DIM * DIM * KK  # 147456
NCORES = 8
SPC = B // NCORES      # samples per core
HP, WP = H + 2, W + 2  # padded 34x34
HALF = HW // 2         # 512 columns per PSUM bank
HROWS = H // 2         # 16 output rows per half

FP = mybir.dt.float32
BF = mybir.dt.bfloat16
BF_NP = ml_dtypes.bfloat16

NWARM = 9              # initial warmup matmuls (HAM ramp, ~3.8us cold)

_BUILT = {}


def build_nc():
    nc = bacc.Bacc("TRN2", target_bir_lowering=False, debug=False,
                   num_devices=NCORES)

    CST = 2 * DIM + 2 * (SPC + 1)  # [pw0 | pw1 | bias(fp32 as 2x bf16)]
    qw = nc.declare_dram_parameter("qw", [SPC, DIM, KK * DIM], BF, isOutput=False)
    xc = nc.declare_dram_parameter("xc", [SPC, DIM, 2 * HW], BF, isOutput=False)
    cst = nc.declare_dram_parameter("cst", [DIM, CST], BF, isOutput=False)
    out = nc.declare_dram_parameter("out", [SPC, DIM, HW], BF, isOutput=True)

    with TileContext(nc) as tc:
        with (
            tc.tile_pool(name="const", bufs=1) as cpool,
            tc.tile_pool(name="wpool", bufs=4) as wpool,
            tc.tile_pool(name="xpool", bufs=5) as xpool,
            tc.tile_pool(name="xppool", bufs=4) as xppool,
            tc.tile_pool(name="opool", bufs=4) as opool,
            tc.tile_pool(name="pp_pool", bufs=4, space="PSUM") as pp_pool,
            tc.tile_pool(name="pc_pool", bufs=4, space="PSUM") as pc_pool,
        ):
            # ── warmup scratch: highest priority so the memset is the very
            # first vector-queue op and warm matmuls start right after the
            # preamble (~7us), overlapping the first input DMAs.
            with tc.tile_wait_until(0.0001):
                warm_sb = cpool.tile([DIM, DIM + HALF], BF)
                # gpsimd's queue preamble ends earliest; warmups can start
                # ~6.6us and the HAM gate flips before the first real matmul
                nc.gpsimd.memset(warm_sb[:], 0.0)
                warm_ps = pc_pool.tile([DIM, HALF], FP, tag="pc")

                def warm(n):
                    for _ in range(n):
                        nc.tensor.matmul(warm_ps[:], lhsT=warm_sb[:, 0:DIM],
                                         rhs=warm_sb[:, DIM:DIM + HALF],
                                         start=True, stop=True)

                warm(NWARM)

            # ── all input DMAs, emitted up-front in consumption order.
            # scalar queue (otherwise idle during fill): cst, xc0-h1, qw0, qw1
            # sync queue: xc0-h0 halves, xc1.., with qw2+ interleaved
            cst_sb = cpool.tile([DIM, CST], BF)
            nc.scalar.dma_start(out=cst_sb[:], in_=cst[:])
            pw_sb = cst_sb[:, 0:2 * DIM]

            xc_sbs, w_sbs = {}, {}

            def load_x(s):
                xc_sb = xpool.tile([DIM, 2 * HW], BF, tag="xc")
                if s == 0:
                    # sample 0 is latency-critical: land blocks in matmul
                    # consumption order (scalar queue must stay clear for
                    # qw0 -- anything ahead of it delays conv0)
                    nc.sync.dma_start(out=xc_sb[:, 0:HALF], in_=xc[s, :, 0:HALF])
                    nc.sync.dma_start(out=xc_sb[:, HALF:HW], in_=xc[s, :, HALF:HW])
                    nc.sync.dma_start(out=xc_sb[:, HW:2 * HW], in_=xc[s, :, HW:2 * HW])
                else:
                    nc.sync.dma_start(out=xc_sb[:], in_=xc[s])
                xc_sbs[s] = xc_sb

            def load_w(s):
                w_sb = wpool.tile([DIM, KK, DIM], BF, tag="w")
                if s <= 1:
                    # split taps 0-5 / 6-8 so conv(s) can start before the
                    # full weight tensor lands (fill is DMA-delivery bound);
                    # rides the small scalar stream, keeping sync xc-only
                    nc.scalar.dma_start(out=w_sb[:, 0:6, :],
                                        in_=qw[s, :, 0:6 * DIM])
                    nc.scalar.dma_start(out=w_sb[:, 6:KK, :],
                                        in_=qw[s, :, 6 * DIM:KK * DIM])
                else:
                    nc.sync.dma_start(out=w_sb[:], in_=qw[s])
                w_sbs[s] = w_sb

            load_x(0)
            load_w(0)
            load_x(1)
            load_w(1)
            load_x(2)
            load_w(2)
            for s in range(3, SPC):
                load_x(s)
                load_w(s)

            def qb_ap(s):
                c = 2 * DIM + 2 * s
                return cst_sb[:, c:c + 2].bitcast(FP)

            pb_ap = cst_sb[:, 2 * DIM + 2 * SPC:2 * DIM + 2 * SPC + 2].bitcast(FP)


            xps = {}

            def proj(s):
                xc_sb = xc_sbs[s]
                xp = xppool.tile([DIM, HP, WP], BF, tag="xp")
                nc.vector.memset(xp[:, 0:1, :], 0.0)
                nc.vector.memset(xp[:, HP - 1:HP, :], 0.0)
                nc.vector.memset(xp[:, 1:HP - 1, 0:1], 0.0)
                nc.vector.memset(xp[:, 1:HP - 1, WP - 1:WP], 0.0)
                for h in range(2):
                    ppt = pp_pool.tile([DIM, HALF], FP, tag="pp")
                    nc.tensor.matmul(ppt[:], lhsT=pw_sb[:, 0:DIM],
                                     rhs=xc_sb[:, 2 * h * HALF:(2 * h + 1) * HALF],
                                     start=True, stop=False)
                    nc.tensor.matmul(ppt[:], lhsT=pw_sb[:, DIM:2 * DIM],
                                     rhs=xc_sb[:, (2 * h + 1) * HALF:(2 * h + 2) * HALF],
                                     start=False, stop=True)
                    dst = xp[:, 1 + HROWS * h:1 + HROWS * (h + 1), 1:1 + W]
                    src = ppt[:].rearrange("p (a b) -> p a b", b=W)
                    # proj evictions live on DVE only: ACT owns conv
                    # evictions + stores, so neither engine's static queue
                    # order inverts across the proj/conv software pipeline
                    nc.vector.tensor_scalar_add(dst, src, pb_ap)
                xps[s] = xp

            def conv(s):
                xp, w_sb = xps[s], w_sbs[s]
                o_sb = opool.tile([DIM, HW], BF, tag="o")
                pct0 = pc_pool.tile([DIM, HALF], FP, tag="pc")
                pct1 = pc_pool.tile([DIM, HALF], FP, tag="pc")
                pcts = [pct0, pct1]
                for h in range(2):
                    for t in range(KK):
                        kh, kw = divmod(t, K)
                        nc.tensor.matmul(
                            pcts[h][:],
                            lhsT=w_sb[:, t, :],
                            rhs=xp[:, HROWS * h + kh:HROWS * (h + 1) + kh,
                                   kw:kw + W],
                            start=(t == 0), stop=(t == KK - 1))
                # conv evictions on ACT; h0 drains while h1 matmuls run
                nc.scalar.activation(
                    o_sb[:, 0:HALF], pct0[:],
                    mybir.ActivationFunctionType.Identity, bias=qb_ap(s))
                if s == SPC - 1:
                    # tail: h0 stores while h1's matmuls still run; h1 is
                    # evicted and stored in two quarter chunks on separate
                    # queues so issue latency and transfer overlap
                    nc.scalar.dma_start(out=out[s, :, 0:HALF],
                                        in_=o_sb[:, 0:HALF])
                    q3 = HALF + HALF // 2
                    nc.vector.tensor_scalar_add(
                        o_sb[:, HALF:q3], pct1[:, 0:HALF // 2], qb_ap(s))
                    nc.sync.dma_start(out=out[s, :, HALF:q3],
                                      in_=o_sb[:, HALF:q3])
                    nc.vector.tensor_scalar_add(
                        o_sb[:, q3:HW], pct1[:, HALF // 2:HALF], qb_ap(s))
                    nc.scalar.dma_start(out=out[s, :, q3:HW],
                                        in_=o_sb[:, q3:HW])
                else:
                    nc.scalar.activation(
                        o_sb[:, HALF:HW], pct1[:],
                        mybir.ActivationFunctionType.Identity, bias=qb_ap(s))
                    nc.scalar.dma_start(out=out[s], in_=o_sb[:])

            # PE phase order, pinned with virtual times (ms units = us*1e-3)
            phases = []
            phases.append((0.0100, lambda: proj(0)))
            phases.append((0.0118, lambda: warm(2)))
            phases.append((0.0125, lambda: conv(0)))
            phases.append((0.0168, lambda: proj(1)))
            phases.append((0.0177, lambda: proj(2)))
            phases.append((0.0184, lambda: conv(1)))
            phases.append((0.0223, lambda: proj(3)))
            phases.append((0.0232, lambda: conv(2)))
            phases.append((0.0271, lambda: proj(4)))
            phases.append((0.0280, lambda: conv(3)))
            phases.append((0.0319, lambda: proj(5)))
            phases.append((0.0328, lambda: conv(4)))
            phases.append((0.0367, lambda: proj(6)))
            phases.append((0.0376, lambda: conv(5)))
            phases.append((0.0415, lambda: proj(7)))
            phases.append((0.0424, lambda: conv(6)))
            phases.append((0.0463, lambda: conv(7)))
            for t_ms, fn in phases:
                with tc.tile_wait_until(t_ms):
                    fn()

    nc.compile()
    return nc


def _prep(question_rep, lhs_rep, rhs_rep, proj_w, proj_b):
    """Host-side shard + layout prep (cheap reshapes/transposes + bf16 cast)."""
    qr = np.ascontiguousarray(question_rep, dtype=np.float32)
    # conv weights: [B, o, i, kh, kw] -> [B, i, (kh kw), o] so each tap is a
    # ready lhsT [i, o] block and the per-sample weight DMA is contiguous
    qw = qr[:, :WDIM].reshape(B, DIM, DIM, K, K).transpose(0, 2, 3, 4, 1)
    qw = np.ascontiguousarray(qw).astype(BF_NP).reshape(B, DIM, KK * DIM)
    qb = np.ascontiguousarray(qr[:, WDIM:])             # [B, 128]
    # per-sample blocks [xl-h0 | xr-h0 | xl-h1 | xr-h1], 512 cols each: each
    # proj half reads one contiguous half of the tensor
    xl = np.asarray(lhs_rep, dtype=np.float32).reshape(B, DIM, 2, HALF)
    xr = np.asarray(rhs_rep, dtype=np.float32).reshape(B, DIM, 2, HALF)
    xcat = np.stack([xl[:, :, 0], xr[:, :, 0], xl[:, :, 1], xr[:, :, 1]],
                    axis=2).reshape(B, DIM, 2 * HW).astype(BF_NP)
    pwt = np.asarray(proj_w, dtype=np.float32).T.astype(BF_NP)  # [256, 128]
    pwm = np.concatenate([pwt[:DIM], pwt[DIM:]], axis=1)        # [128, 256]
    pb = np.asarray(proj_b, dtype=np.float32).reshape(DIM, 1)

    in_maps = []
    for c in range(NCORES):
        sl = slice(c * SPC, (c + 1) * SPC)
        biasm = np.concatenate([qb[sl].T, pb], axis=1).astype(np.float32)
        # bias fp32 bits ride as pairs of bf16 columns after pw
        bias_bf = np.ascontiguousarray(biasm).view(np.uint16).view(BF_NP)
        cstm = np.concatenate([pwm, bias_bf], axis=1)  # [128, 256+18]
        in_maps.append({
            "qw": np.ascontiguousarray(qw[sl]),
            "xc": np.ascontiguousarray(xcat[sl]),
            "cst": np.ascontiguousarray(cstm),
        })
    return in_maps


def kernel(question_rep, lhs_rep, rhs_rep, proj_w, proj_b, _run_kwargs=None):
    if "nc" not in _BUILT:
        _BUILT["nc"] = build_nc()
    nc = _BUILT["nc"]
    in_maps = _prep(question_rep, lhs_rep, rhs_rep, proj_w, proj_b)
    res = run_bass_kernel_spmd(nc, in_maps, core_ids=list(range(NCORES)),
                               **(_run_kwargs or {}))
    out = np.concatenate(
        [np.asarray(res.results[c]["out"]) for c in range(NCORES)], axis=0)
    if _run_kwargs is not None:
        _BUILT["last_result"] = res
    return out.astype(np.float32).reshape(B, DIM, H, W)


if __name__ == "__main__":
    rng = np.random.default_rng(0)
    inputs = {
        "question_rep": rng.standard_normal((B, WDIM + DIM), dtype=np.float32) * 0.05,
        "lhs_rep": rng.standard_normal((B, DIM, H, W), dtype=np.float32),
        "rhs_rep": rng.standard_normal((B, DIM, H, W), dtype=np.float32),
        "proj_w": rng.standard_normal((DIM, 2 * DIM), dtype=np.float32),
        "proj_b": rng.standard_normal((DIM,), dtype=np.float32) * 0.01,
    }
    out = kernel(**inputs)
    print("ran, out shape:", out.shape)
